# revision 27
# baseline (speedup 1.0000x reference)
"""GCN layer on 8 TRN2 NeuronCores (raw Bass, no Tile framework).

Computation (matches the reference):
    support  = x @ weight                          # [N, F]
    A        = scatter(adj, edge_w) + I            # dense [N, N], duplicate edges sum
    deg      = A.sum(axis=1)
    dis      = 1/sqrt(deg + 1e-10)
    out      = (dis[:,None] * A * dis[None,:]) @ support + bias

Strategy (v2, fp8 streaming):
  * Host folds the degree normalization into the dense adjacency and also
    precomputes support = x @ weight (268 MFLOP - trivial next to the 17 GFLOP
    propagation, which stays on device).
  * The dense A'^T = normalized adjacency transposed is stored in fp8 e3m4
    (1 byte/elt) with a per-row power-of-2 scale c_j chosen so each row's max
    lands in [7.75, 15.5] (e3m4 normal range top) - measured end-to-end rel
    err 1.31e-2 (1.37e-2 even if the PE flushes fp8 denormals), vs the 2e-2
    gate.  The inverse scales fold exactly into the support rows on the host
    (any per-j scale cancels in the contraction), so the device kernel is a
    single big matmul.
  * Row-shard the propagation across 8 cores (1024 output rows each):
      out^T[f, i] = sum_j S'[j, f] * A''[j, i],  S' = (x@W) / c,  A'' = c * A'^T
    64 j-tiles of K=128, accumulated into two PSUM banks (i halves of 512).
  * Per core DMA: 8 MB fp8 A + 2 MB bf16 S + 0.25 MB out ~= 10.3 MB, vs the
    ~358 GB/s HBM/NC limit -> ~29 us; PE: 65536 moving columns @2.4 GHz ->
    ~27.5 us.  Both engines run at the roofline "ridge" with S and A tiles
    interleaved j-ordered in one HWDGE queue so the PE starts ~0.7 us in and
    is never starved.  A couple of warm-up matmuls on scratch start the HAM
    clock-gate release early.
"""

from contextlib import ExitStack

import numpy as np
import ml_dtypes

N = 8192
F = 128
NCORES = 8
RPC = N // NCORES  # 1024 output rows per core
JT = N // 128  # 64 contraction tiles
EPS = 1e-10
F8MAX = 15.5  # e3m4 max normal

# Input DMA plan: A tiles (fp8, 128 KB each) stream alone on the sync HWDGE
# ring; S tiles (bf16, 32 KB each) + bias go on the scalar ring.  The two
# rings' SDMA packets round-robin, so per-DMA completion overheads on one
# ring overlap the other ring's transfers, and S trickles in alongside A.
# Every chunk gets its OWN semaphore: a DMA's 16 completion increments come
# from the 16 SDMA engines independently, so a shared cumulative counter
# does NOT imply earlier DMAs in the ring finished.
# A chunks: small first (PE starts early) / large middle (amortize per-DMA
# overhead) / small last (tail stays just-in-time and epilogue starts early).
ACHUNKS = [(0, 1), (1, 2), (3, 2), (5, 3)] + [
    (8 + 4 * i, 4) for i in range(13)
] + [(60, 2), (62, 1), (63, 1)]
SCHUNKS = [(0, 2), (2, 6), (8, 8), (16, 16), (32, 16), (48, 16)]
# before the matmuls of j-tile jt, also wait for this S chunk index
SWAIT = {0: 0, 2: 1, 8: 2, 16: 3, 32: 4, 48: 5}

_graph_cache = {}


def _build_graph():
    from concourse import bacc, mybir

    nc = bacc.Bacc("TRN2", target_bir_lowering=False, debug=False, num_devices=NCORES)
    # Partition-major layouts: at[p, jt, i] = A''[jt*128 + p, i], etc., so each
    # SBUF partition line is one contiguous DRAM read.
    at = nc.declare_dram_parameter("at", [F, JT, RPC], mybir.dt.float8e3, isOutput=False)
    sp = nc.declare_dram_parameter("sp", [F, JT, F], mybir.dt.bfloat16, isOutput=False)
    bias = nc.declare_dram_parameter("bias", [F, 1], mybir.dt.float32, isOutput=False)
    out = nc.declare_dram_parameter("out", [F, RPC], mybir.dt.bfloat16, isOutput=True)

    with ExitStack() as ctx:
        e = ctx.enter_context
        a_sb = e(nc.sbuf_tensor("a_sb", [F, JT, RPC], mybir.dt.float8e3))
        s_sb = e(nc.sbuf_tensor("s_sb", [F, JT, F], mybir.dt.bfloat16))
        bias_sb = e(nc.sbuf_tensor("bias_sb", [F, 1], mybir.dt.float32))
        out_sb = e(nc.sbuf_tensor("out_sb", [F, RPC], mybir.dt.bfloat16))
        # scratch for HAM warm-up matmuls; contents irrelevant
        warm_sb = e(nc.sbuf_tensor("warm_sb", [F, 128], mybir.dt.bfloat16))

        pp0 = e(nc.psum_tensor("pp0", [F, 512], mybir.dt.float32))
        pp1 = e(nc.psum_tensor("pp1", [F, 512], mybir.dt.float32))
        pwarm = e(nc.psum_tensor("pwarm", [F, 512], mybir.dt.float32))

        biasld = e(nc.semaphore("biasld"))
        asem = [e(nc.semaphore(f"asem{i}")) for i in range(len(ACHUNKS))]
        ssem = [e(nc.semaphore(f"ssem{i}")) for i in range(len(SCHUNKS))]
        pp0done = e(nc.semaphore("pp0done"))
        pp1done = e(nc.semaphore("pp1done"))
        biassem = e(nc.semaphore("biassem"))
        outsem = e(nc.semaphore("outsem"))
        warmsem = e(nc.semaphore("warmsem"))
        ep1sem = e(nc.semaphore("ep1sem"))

        with nc.Block(no_gpsimd_drain=True) as block:

            @block.sync
            def _(sync):
                for ci, (j0, n) in enumerate(ACHUNKS):
                    sync.dma_start(
                        a_sb[:, j0 : j0 + n, :], at[:, j0 : j0 + n, :]
                    ).then_inc(asem[ci], 16)
                sync.dma_start(out[:, 0:512], out_sb[:, 0:512]).then_inc(
                    outsem, 16
                )._wait_ge(biassem, 1)
                sync.wait_ge(outsem, 32)

            @block.scalar
            def _(scalar):
                # S0 first: it gates the PE's first matmul; bias is only
                # needed by the epilogue
                for ci, (j0, n) in enumerate(SCHUNKS):
                    scalar.dma_start(
                        s_sb[:, j0 : j0 + n, :], sp[:, j0 : j0 + n, :]
                    ).then_inc(ssem[ci], 16)
                    if ci == 0:
                        scalar.dma_start(bias_sb[:], bias[:]).then_inc(biasld, 16)
                # second output half on this ring so the two store issue
                # latencies and completion receipts overlap
                scalar.dma_start(out[:, 512:1024], out_sb[:, 512:1024]).then_inc(
                    outsem, 16
                )._wait_ge(ep1sem, 1)

            @block.tensor
            def _(tensor):
                # HAM warm-up: PE busy almost from t=0 so the 2.4 GHz
                # un-throttle fires earlier; results land in scratch.
                tensor.wait_ge(warmsem, 1)
                for _ in range(2):
                    nc.tensor.matmul(
                        pwarm[:, 0:128], warm_sb[:], warm_sb[:],
                        start=True, stop=True,
                    )
                for ch, (j0, n) in enumerate(ACHUNKS[:-3]):
                    tensor.wait_ge(asem[ch], 16)
                    for t in range(n):
                        jt = j0 + t
                        if jt in SWAIT:
                            tensor.wait_ge(ssem[SWAIT[jt]], 16)
                        first = jt == 0
                        s_tile = s_sb[:, jt, :]
                        nc.tensor.matmul(
                            pp0[:], s_tile, a_sb[:, jt, 0:512],
                            start=first, stop=False,
                        )
                        nc.tensor.matmul(
                            pp1[:], s_tile, a_sb[:, jt, 512:1024],
                            start=first, stop=False,
                        )
                # tail (tiles 60..63): finish all of pp0 first so the
                # low-half epilogue (DVE bias-add + store) overlaps pp1's
                # last matmuls
                nchunks = len(ACHUNKS)
                tensor.wait_ge(asem[nchunks - 3], 16)
                nc.tensor.matmul(
                    pp0[:], s_sb[:, 60, :], a_sb[:, 60, 0:512],
                    start=False, stop=False,
                )
                nc.tensor.matmul(
                    pp0[:], s_sb[:, 61, :], a_sb[:, 61, 0:512],
                    start=False, stop=False,
                )
                tensor.wait_ge(asem[nchunks - 2], 16)
                nc.tensor.matmul(
                    pp0[:], s_sb[:, 62, :], a_sb[:, 62, 0:512],
                    start=False, stop=False,
                )
                tensor.wait_ge(asem[nchunks - 1], 16)
                nc.tensor.matmul(
                    pp0[:], s_sb[:, 63, :], a_sb[:, 63, 0:512],
                    start=False, stop=True,
                ).then_inc(pp0done)
                for jt in (60, 61, 62):
                    nc.tensor.matmul(
                        pp1[:], s_sb[:, jt, :], a_sb[:, jt, 512:1024],
                        start=False, stop=False,
                    )
                nc.tensor.matmul(
                    pp1[:], s_sb[:, 63, :], a_sb[:, 63, 512:1024],
                    start=False, stop=True,
                ).then_inc(pp1done)

            @block.vector
            def _(vector):
                nc.vector.memset(warm_sb[:], 1.0).then_inc(warmsem)
                vector.wait_ge(biasld, 16)  # bias arrived
                nc.vector.tensor_scalar_add(
                    out_sb[:, 0:512], pp0[:], bias_sb[:]
                ).then_inc(biassem)._wait_ge(pp0done, 1)
                nc.vector.tensor_scalar_add(
                    out_sb[:, 512:1024], pp1[:], bias_sb[:]
                ).then_inc(ep1sem)._wait_ge(pp1done, 1)

    nc.compile()
    return nc


def _get_graph():
    if "nc" not in _graph_cache:
        _graph_cache["nc"] = _build_graph()
    return _graph_cache["nc"]


def _prepare_in_maps(x, adj, edge_w, weight, bias):
    x = np.asarray(x, dtype=np.float32)
    adj = np.asarray(adj).astype(np.int64)
    edge_w = np.asarray(edge_w, dtype=np.float32)
    weight = np.asarray(weight, dtype=np.float32)
    bias = np.asarray(bias, dtype=np.float32)

    rows, cols = adj[0], adj[1]
    deg = 1.0 + np.bincount(rows, weights=edge_w.astype(np.float64), minlength=N)
    dis = (1.0 / np.sqrt(deg + EPS)).astype(np.float32)

    # A'^T[c, r] = dis[r] * w_e * dis[c]; diagonal gets dis[i]^2 (self loop).
    vals = edge_w * dis[rows] * dis[cols]
    at = np.zeros((N, N), dtype=np.float32)
    np.add.at(at, (cols, rows), vals)
    idx = np.arange(N)
    at[idx, idx] += dis * dis

    # per-row (j = contraction index) power-of-2 scale into e3m4 normal range;
    # the inverse folds exactly into the support rows
    rowmax = at.max(axis=1)
    c = np.exp2(np.floor(np.log2(F8MAX / rowmax))).astype(np.float32)
    atq = (at * c[:, None]).astype(ml_dtypes.float8_e3m4)

    support = (x @ weight) / c[:, None]
    # partition-major S': [8192, 128] -> [128, 64, 128]
    spb = np.ascontiguousarray(
        support.astype(ml_dtypes.bfloat16).reshape(JT, F, F).transpose(1, 0, 2)
    )
    bias_col = np.ascontiguousarray(bias.reshape(F, 1))

    return [
        {
            # [8192, RPC] fp8 shard -> partition-major [128, 64, RPC]
            "at": np.ascontiguousarray(
                atq[:, c_id * RPC : (c_id + 1) * RPC]
                .reshape(JT, F, RPC)
                .transpose(1, 0, 2)
            ),
            "sp": spb,
            "bias": bias_col,
        }
        for c_id in range(NCORES)
    ]


def _run(in_maps, trace=False, tmpdir=None):
    from concourse.bass_utils import run_bass_kernel_spmd

    nc = _get_graph()
    return run_bass_kernel_spmd(
        nc, in_maps, core_ids=list(range(NCORES)), trace=trace, tmpdir=tmpdir
    )


def _assemble(results):
    return np.ascontiguousarray(
        np.concatenate([results[c]["out"].T for c in range(NCORES)], axis=0)
    ).astype(np.float32)


def kernel(x, adj, edge_w, weight, bias):
    in_maps = _prepare_in_maps(x, adj, edge_w, weight, bias)
    res = _run(in_maps, trace=False)
    return _assemble(res.results)


def kernel_traced(x, adj, edge_w, weight, bias, tmpdir=None):
    """Same as kernel() but profiles the NEFF; returns (output, BassKernelResults)."""
    in_maps = _prepare_in_maps(x, adj, edge_w, weight, bias)
    res = _run(in_maps, trace=True, tmpdir=tmpdir)
    return _assemble(res.results), res


# revision 29
# speedup vs baseline: 1.3248x; 1.3248x over previous
"""GCN layer on 8 TRN2 NeuronCores (raw Bass, no Tile framework).

Computation (matches the reference):
    support  = x @ weight                          # [N, F]
    A        = scatter(adj, edge_w) + I            # dense [N, N], duplicate edges sum
    deg      = A.sum(axis=1)
    dis      = 1/sqrt(deg + 1e-10)
    out      = (dis[:,None] * A * dis[None,:]) @ support + bias

Strategy (v2, fp8 streaming):
  * Host folds the degree normalization into the dense adjacency and also
    precomputes support = x @ weight (268 MFLOP - trivial next to the 17 GFLOP
    propagation, which stays on device).
  * The dense A'^T = normalized adjacency transposed is stored in fp8 e3m4
    (1 byte/elt) with a per-row power-of-2 scale c_j chosen so each row's max
    lands in [7.75, 15.5] (e3m4 normal range top) - measured end-to-end rel
    err 1.31e-2 (1.37e-2 even if the PE flushes fp8 denormals), vs the 2e-2
    gate.  The inverse scales fold exactly into the support rows on the host
    (any per-j scale cancels in the contraction), so the device kernel is a
    single big matmul.
  * Row-shard the propagation across 8 cores (1024 output rows each):
      out^T[f, i] = sum_j S'[j, f] * A''[j, i],  S' = (x@W) / c,  A'' = c * A'^T
    64 j-tiles of K=128, accumulated into two PSUM banks (i halves of 512).
  * Per core DMA: 8 MB fp8 A + 2 MB bf16 S + 0.25 MB out ~= 10.3 MB, vs the
    ~358 GB/s HBM/NC limit -> ~29 us; PE: 65536 moving columns @2.4 GHz ->
    ~27.5 us.  Both engines run at the roofline "ridge" with S and A tiles
    interleaved j-ordered in one HWDGE queue so the PE starts ~0.7 us in and
    is never starved.  A couple of warm-up matmuls on scratch start the HAM
    clock-gate release early.
"""

from contextlib import ExitStack

import numpy as np
import ml_dtypes

N = 8192
F = 128
NCORES = 8
RPC = N // NCORES  # 1024 output rows per core
JT = N // 128  # 64 contraction tiles
EPS = 1e-10
F8MAX = 15.5  # e3m4 max normal

# Input DMA plan: A tiles (fp8, 128 KB each) stream alone on the sync HWDGE
# ring; S tiles (bf16, 32 KB each) + bias go on the scalar ring.  The two
# rings' SDMA packets round-robin, so per-DMA completion overheads on one
# ring overlap the other ring's transfers, and S trickles in alongside A.
# Every chunk gets its OWN semaphore: a DMA's 16 completion increments come
# from the 16 SDMA engines independently, so a shared cumulative counter
# does NOT imply earlier DMAs in the ring finished.
# A chunks: small first (PE starts early) / large middle (amortize per-DMA
# overhead) / small last (tail stays just-in-time and epilogue starts early).
ACHUNKS = [(0, 1), (1, 2), (3, 2), (5, 3)] + [
    (8 + 4 * i, 4) for i in range(13)
] + [(60, 2), (62, 1), (63, 1)]
SCHUNKS = [(0, 2), (2, 6), (8, 8), (16, 16), (32, 16), (48, 16)]
# before the matmuls of j-tile jt, also wait for this S chunk index
SWAIT = {0: 0, 2: 1, 8: 2, 16: 3, 32: 4, 48: 5}

_graph_cache = {}


def _build_graph():
    from concourse import bacc, mybir

    nc = bacc.Bacc("TRN2", target_bir_lowering=False, debug=False, num_devices=NCORES)
    # Partition-major layouts: at[p, jt, i] = A''[jt*128 + p, i], etc., so each
    # SBUF partition line is one contiguous DRAM read.
    at = nc.declare_dram_parameter("at", [F, JT, RPC], mybir.dt.float8e3, isOutput=False)
    sp = nc.declare_dram_parameter("sp", [F, JT, F], mybir.dt.bfloat16, isOutput=False)
    bias = nc.declare_dram_parameter("bias", [F, 1], mybir.dt.float32, isOutput=False)
    out = nc.declare_dram_parameter("out", [F, RPC], mybir.dt.bfloat16, isOutput=True)

    with ExitStack() as ctx:
        e = ctx.enter_context
        a_sb = e(nc.sbuf_tensor("a_sb", [F, JT, RPC], mybir.dt.float8e3))
        s_sb = e(nc.sbuf_tensor("s_sb", [F, JT, F], mybir.dt.bfloat16))
        bias_sb = e(nc.sbuf_tensor("bias_sb", [F, 1], mybir.dt.float32))
        out_sb = e(nc.sbuf_tensor("out_sb", [F, RPC], mybir.dt.bfloat16))
        # scratch for HAM warm-up matmuls; contents irrelevant
        warm_sb = e(nc.sbuf_tensor("warm_sb", [F, 128], mybir.dt.bfloat16))

        pp0 = e(nc.psum_tensor("pp0", [F, 512], mybir.dt.float32))
        pp1 = e(nc.psum_tensor("pp1", [F, 512], mybir.dt.float32))
        pwarm = e(nc.psum_tensor("pwarm", [F, 512], mybir.dt.float32))

        biasld = e(nc.semaphore("biasld"))
        asem = [e(nc.semaphore(f"asem{i}")) for i in range(len(ACHUNKS))]
        ssem = [e(nc.semaphore(f"ssem{i}")) for i in range(len(SCHUNKS))]
        pp0done = e(nc.semaphore("pp0done"))
        pp1done = e(nc.semaphore("pp1done"))
        biassem = e(nc.semaphore("biassem"))
        outsem = e(nc.semaphore("outsem"))
        warmsem = e(nc.semaphore("warmsem"))
        ep1sem = e(nc.semaphore("ep1sem"))

        with nc.Block(no_gpsimd_drain=True) as block:

            @block.sync
            def _(sync):
                for ci, (j0, n) in enumerate(ACHUNKS):
                    sync.dma_start(
                        a_sb[:, j0 : j0 + n, :], at[:, j0 : j0 + n, :]
                    ).then_inc(asem[ci], 16)
                sync.dma_start(out[:, 0:512], out_sb[:, 0:512]).then_inc(
                    outsem, 16
                )._wait_ge(biassem, 1)
                sync.dma_start(out[:, 768:1024], out_sb[:, 768:1024]).then_inc(
                    outsem, 16
                )._wait_ge(ep1sem, 1)
                sync.wait_ge(outsem, 48)

            @block.scalar
            def _(scalar):
                # S0 first: it gates the PE's first matmul; bias is only
                # needed by the epilogue
                for ci, (j0, n) in enumerate(SCHUNKS):
                    scalar.dma_start(
                        s_sb[:, j0 : j0 + n, :], sp[:, j0 : j0 + n, :]
                    ).then_inc(ssem[ci], 16)
                    if ci == 0:
                        scalar.dma_start(bias_sb[:], bias[:]).then_inc(biasld, 16)
                # high-half store split across both rings so the two
                # completion receipts overlap
                scalar.dma_start(out[:, 512:768], out_sb[:, 512:768]).then_inc(
                    outsem, 16
                )._wait_ge(ep1sem, 1)

            @block.tensor
            def _(tensor):
                # HAM warm-up: PE busy almost from t=0 so the 2.4 GHz
                # un-throttle fires earlier; results land in scratch.
                tensor.wait_ge(warmsem, 1)
                for _ in range(2):
                    nc.tensor.matmul(
                        pwarm[:, 0:128], warm_sb[:], warm_sb[:],
                        start=True, stop=True,
                    )
                for ch, (j0, n) in enumerate(ACHUNKS[:-3]):
                    tensor.wait_ge(asem[ch], 16)
                    for t in range(n):
                        jt = j0 + t
                        if jt in SWAIT:
                            tensor.wait_ge(ssem[SWAIT[jt]], 16)
                        first = jt == 0
                        s_tile = s_sb[:, jt, :]
                        nc.tensor.matmul(
                            pp0[:], s_tile, a_sb[:, jt, 0:512],
                            start=first, stop=False,
                        )
                        nc.tensor.matmul(
                            pp1[:], s_tile, a_sb[:, jt, 512:1024],
                            start=first, stop=False,
                        )
                # tail (tiles 60..63): finish all of pp0 first so the
                # low-half epilogue (DVE bias-add + store) overlaps pp1's
                # last matmuls
                nchunks = len(ACHUNKS)
                tensor.wait_ge(asem[nchunks - 3], 16)
                nc.tensor.matmul(
                    pp0[:], s_sb[:, 60, :], a_sb[:, 60, 0:512],
                    start=False, stop=False,
                )
                nc.tensor.matmul(
                    pp0[:], s_sb[:, 61, :], a_sb[:, 61, 0:512],
                    start=False, stop=False,
                )
                tensor.wait_ge(asem[nchunks - 2], 16)
                nc.tensor.matmul(
                    pp0[:], s_sb[:, 62, :], a_sb[:, 62, 0:512],
                    start=False, stop=False,
                )
                tensor.wait_ge(asem[nchunks - 1], 16)
                nc.tensor.matmul(
                    pp0[:], s_sb[:, 63, :], a_sb[:, 63, 0:512],
                    start=False, stop=True,
                ).then_inc(pp0done)
                for jt in (60, 61, 62):
                    nc.tensor.matmul(
                        pp1[:], s_sb[:, jt, :], a_sb[:, jt, 512:1024],
                        start=False, stop=False,
                    )
                nc.tensor.matmul(
                    pp1[:], s_sb[:, 63, :], a_sb[:, 63, 512:1024],
                    start=False, stop=True,
                ).then_inc(pp1done)

            @block.vector
            def _(vector):
                nc.vector.memset(warm_sb[:], 1.0).then_inc(warmsem)
                vector.wait_ge(biasld, 16)  # bias arrived
                nc.vector.tensor_scalar_add(
                    out_sb[:, 0:512], pp0[:], bias_sb[:]
                ).then_inc(biassem)._wait_ge(pp0done, 1)
                nc.vector.tensor_scalar_add(
                    out_sb[:, 512:1024], pp1[:], bias_sb[:]
                ).then_inc(ep1sem)._wait_ge(pp1done, 1)

    nc.compile()
    return nc


def _get_graph():
    if "nc" not in _graph_cache:
        _graph_cache["nc"] = _build_graph()
    return _graph_cache["nc"]


def _prepare_in_maps(x, adj, edge_w, weight, bias):
    x = np.asarray(x, dtype=np.float32)
    adj = np.asarray(adj).astype(np.int64)
    edge_w = np.asarray(edge_w, dtype=np.float32)
    weight = np.asarray(weight, dtype=np.float32)
    bias = np.asarray(bias, dtype=np.float32)

    rows, cols = adj[0], adj[1]
    deg = 1.0 + np.bincount(rows, weights=edge_w.astype(np.float64), minlength=N)
    dis = (1.0 / np.sqrt(deg + EPS)).astype(np.float32)

    # A'^T[c, r] = dis[r] * w_e * dis[c]; diagonal gets dis[i]^2 (self loop).
    vals = edge_w * dis[rows] * dis[cols]
    at = np.zeros((N, N), dtype=np.float32)
    np.add.at(at, (cols, rows), vals)
    idx = np.arange(N)
    at[idx, idx] += dis * dis

    # per-row (j = contraction index) power-of-2 scale into e3m4 normal range;
    # the inverse folds exactly into the support rows
    rowmax = at.max(axis=1)
    c = np.exp2(np.floor(np.log2(F8MAX / rowmax))).astype(np.float32)
    atq = (at * c[:, None]).astype(ml_dtypes.float8_e3m4)

    support = (x @ weight) / c[:, None]
    # partition-major S': [8192, 128] -> [128, 64, 128]
    spb = np.ascontiguousarray(
        support.astype(ml_dtypes.bfloat16).reshape(JT, F, F).transpose(1, 0, 2)
    )
    bias_col = np.ascontiguousarray(bias.reshape(F, 1))

    return [
        {
            # [8192, RPC] fp8 shard -> partition-major [128, 64, RPC]
            "at": np.ascontiguousarray(
                atq[:, c_id * RPC : (c_id + 1) * RPC]
                .reshape(JT, F, RPC)
                .transpose(1, 0, 2)
            ),
            "sp": spb,
            "bias": bias_col,
        }
        for c_id in range(NCORES)
    ]


def _run(in_maps, trace=False, tmpdir=None):
    from concourse.bass_utils import run_bass_kernel_spmd

    nc = _get_graph()
    return run_bass_kernel_spmd(
        nc, in_maps, core_ids=list(range(NCORES)), trace=trace, tmpdir=tmpdir
    )


def _assemble(results):
    return np.ascontiguousarray(
        np.concatenate([results[c]["out"].T for c in range(NCORES)], axis=0)
    ).astype(np.float32)


def kernel(x, adj, edge_w, weight, bias):
    in_maps = _prepare_in_maps(x, adj, edge_w, weight, bias)
    res = _run(in_maps, trace=False)
    return _assemble(res.results)


def kernel_traced(x, adj, edge_w, weight, bias, tmpdir=None):
    """Same as kernel() but profiles the NEFF; returns (output, BassKernelResults)."""
    in_maps = _prepare_in_maps(x, adj, edge_w, weight, bias)
    res = _run(in_maps, trace=True, tmpdir=tmpdir)
    return _assemble(res.results), res
